# revision 9
# baseline (speedup 1.0000x reference)
"""GAT 2-layer kernel for TRN2, 8 NeuronCores.

Dst-sharded edge-parallel design:
  - Nodes partitioned across 8 cores; per core sorted into degree classes
    (padded in-degree D incl self-loop); class tiles hold m=floor(128/D)
    nodes x D slots = up to 128 slots, slot p -> node p//D.
  - Per-edge rows gathered by src from a replicated fp16 feature table
    (AllGather per layer); attention terms computed per slot; aggregation on
    the TensorEngine: lhsT=MSG[slots,ch] x rhs=ones[slots,m] accumulates
    transposed psum [ch, node-cols]. Per-tile `ones` matrices are host data
    and also mask pad slots / dummy nodes (no sentinel rows needed).
  - Softmax without max-subtraction (|e| <~ 8 by construction); the
    denominator divides after aggregation.
  - W2 and the a2 row projections fold into the L1 epilogue matmul, so the
    L2 table rows are [h2@W2 (64) | asrc2 | pad].
"""
import sys
import numpy as np

sys.path.insert(0, "/opt/trn_rl_repo")

N = 100000
E = 1600000
IN = 128
H1, C1 = 8, 16
D1 = H1 * C1  # 128
OUT = 64
NCORES = 8
NPC_REAL = 12500
EW1 = 136            # L1 table row: 128 ch (c,h) + 8 asrc
EW2 = 68             # L2 table row: 64 ch + asrc2 + 3 pad
KB = 32              # tiles per gather batch
NEG_SLOPE = 0.2

LEVELS = np.array([2, 3, 4, 5, 6, 7, 8, 9, 10, 12, 14, 16, 18, 21, 25, 32,
                   42, 64, 128])


def _build_structure(edge_index):
    src = np.concatenate([edge_index[0].astype(np.int64),
                          np.arange(N, dtype=np.int64)])
    dst = np.concatenate([edge_index[1].astype(np.int64),
                          np.arange(N, dtype=np.int64)])
    deg = np.bincount(dst, minlength=N)
    assert deg.max() <= LEVELS[-1]
    lvl_of = np.searchsorted(LEVELS, deg)

    counts = np.zeros((NCORES, len(LEVELS)), dtype=np.int64)
    for c in range(NCORES):
        lo, hi = c * NPC_REAL, min((c + 1) * NPC_REAL, N)
        counts[c] = np.bincount(lvl_of[lo:hi], minlength=len(LEVELS))
    cls_nodes = counts.max(axis=0)

    sched = []
    node_base = 0
    for li, D in enumerate(LEVELS):
        n_c = int(cls_nodes[li])
        if n_c == 0:
            continue
        D = int(D)
        m = 128 // D
        ntiles = -(-n_c // m)
        sched.append(dict(D=D, m=m, ntiles=ntiles, node_base=node_base,
                          n_nodes=n_c))
        node_base += ntiles * m
    npad = -(-node_base // 512) * 512
    ntiles_tot = sum(s["ntiles"] for s in sched)
    nslots = ntiles_tot * 128

    groups = []
    tile_base = 0
    for s in sched:
        m = s["m"]
        left = s["ntiles"]
        g_tiles = max(1, 128 // m)
        tb = 0
        while left > 0:
            t = min(g_tiles, left)
            groups.append(dict(tile_base=tile_base + tb, ntiles=t, m=m,
                               ncols=t * m, node_base=s["node_base"] + tb * m))
            tb += t
            left -= t
        tile_base += s["ntiles"]

    ms = []
    for g in groups:
        ms.extend([g["m"]] * g["ntiles"])
    col_of_tile = np.zeros(ntiles_tot + 1, dtype=np.int64)
    np.cumsum(ms, out=col_of_tile[1:])

    # per-core permutation
    per_core = []
    perm_global = np.empty(N, dtype=np.int64)
    for c in range(NCORES):
        lo, hi = c * NPC_REAL, min((c + 1) * NPC_REAL, N)
        nodes_c = np.arange(lo, hi)
        order = np.argsort(lvl_of[lo:hi], kind="stable")
        nodes_sorted = nodes_c[order]
        lv_sorted = lvl_of[nodes_sorted]
        perm_rows = np.full(npad, -1, dtype=np.int64)
        for s in sched:
            li = int(np.searchsorted(LEVELS, s["D"]))
            sel = nodes_sorted[lv_sorted == li]
            perm_rows[s["node_base"]: s["node_base"] + len(sel)] = sel
        per_core.append(dict(perm_rows=perm_rows))
        valid = perm_rows >= 0
        perm_global[perm_rows[valid]] = np.where(valid)[0] + c * npad

    # dst-sorted edge lists
    order_e = np.argsort(dst, kind="stable")
    src_s, dst_s = src[order_e], dst[order_e]
    node_ptr = np.zeros(N + 1, dtype=np.int64)
    np.cumsum(np.bincount(dst_s, minlength=N), out=node_ptr[1:])

    for c in range(NCORES):
        perm_rows = per_core[c]["perm_rows"]
        src_idx = np.zeros(nslots, dtype=np.int64)
        valid = np.zeros(nslots, dtype=bool)
        slot = 0
        for s in sched:
            D, m = s["D"], s["m"]
            for t in range(s["ntiles"]):
                for j in range(m):
                    n = perm_rows[s["node_base"] + t * m + j]
                    if n >= 0:
                        lo_, hi_ = node_ptr[n], node_ptr[n + 1]
                        es = src_s[lo_:hi_]
                        base = slot + j * D
                        src_idx[base: base + len(es)] = perm_global[es]
                        valid[base: base + len(es)] = True
                slot += 128
        per_core[c]["src_idx"] = src_idx
        per_core[c]["valid"] = valid

    return dict(sched=sched, groups=groups, ntiles=ntiles_tot, nslots=nslots,
                npad=npad, per_core=per_core, col_of_tile=col_of_tile)


def _host_arrays(st, x, W1, a1_src, a1_dst, b1, W2, a2_src, a2_dst, b2):
    f16, f32 = np.float16, np.float32
    perm_ch = np.array([h * C1 + c for c in range(C1) for h in range(H1)])
    W1f = np.asarray(W1, f32)
    W1p = W1f[:, perm_ch].copy()
    A1s = np.zeros((D1, H1), f32)
    A1d = np.zeros((D1, H1), f32)
    for h in range(H1):
        A1s[h * C1:(h + 1) * C1, h] = np.asarray(a1_src, f32)[h]
        A1d[h * C1:(h + 1) * C1, h] = np.asarray(a1_dst, f32)[h]
    W1aux = np.concatenate([W1f @ A1s, W1f @ A1d], axis=1)
    W2p = np.asarray(W2, f32)[perm_ch, :]
    a2s = np.asarray(a2_src, f32).reshape(OUT)
    a2d = np.asarray(a2_dst, f32).reshape(OUT)
    W2all = np.concatenate([W2p, (W2p @ a2s)[:, None], (W2p @ a2d)[:, None]],
                           axis=1).astype(f16)
    b1p = np.asarray(b1, f32)[perm_ch].reshape(128, 1).copy()
    b2c = np.asarray(b2, f32).reshape(64, 1)
    exp8 = np.zeros((8, 128), f32)
    for ch in range(128):
        exp8[ch % 8, ch] = 1.0
    ones1 = np.ones((1, OUT), f32)

    npad, ntiles = st["npad"], st["ntiles"]
    col_of_tile = st["col_of_tile"]
    ones_cols = int(col_of_tile[-1])

    per_core_arrays = []
    for c in range(NCORES):
        pc = st["per_core"][c]
        perm_rows = pc["perm_rows"]
        valid_n = perm_rows >= 0
        xT = np.zeros((IN, npad), f32)
        xT[:, valid_n] = np.asarray(x, f32)[perm_rows[valid_n]].T
        idx_pt = pc["src_idx"].reshape(ntiles, 128).T.astype(np.int32).copy()

        ones_pt = np.zeros((128, ones_cols), f16)
        vv = pc["valid"].reshape(ntiles, 128)
        tile_i = 0
        p = np.arange(128)
        for g in st["groups"]:
            m = g["m"]
            D = 128 // m
            jj = p // D
            live = jj < m
            for t in range(g["ntiles"]):
                cb = int(col_of_tile[tile_i])
                sel = vv[tile_i] & live
                ones_pt[p[sel], cb + jj[sel]] = 1.0
                tile_i += 1
        per_core_arrays.append(dict(xT=xT, idx_pt=idx_pt, ones_pt=ones_pt))

    consts = dict(W1p=W1p, W1aux=W1aux, W2all=W2all, b1p=b1p, b2c=b2c,
                  exp8=exp8, ones1=ones1)
    return consts, per_core_arrays


def _build_kernel(st):
    import concourse.bass as bass
    import concourse.bacc as bacc
    import concourse.tile as tile
    from concourse import mybir
    from contextlib import ExitStack

    f16, f32, i32 = mybir.dt.float16, mybir.dt.float32, mybir.dt.int32
    AF = mybir.ActivationFunctionType
    OP = mybir.AluOpType

    npad = st["npad"]
    ntiles = st["ntiles"]
    nslots = st["nslots"]
    col_of_tile = st["col_of_tile"]
    ones_cols = int(col_of_tile[-1])
    groups = st["groups"]
    NT1 = NCORES * npad

    nc = bacc.Bacc("TRN2", target_bir_lowering=False, debug=False,
                   num_devices=NCORES)

    xT = nc.declare_dram_parameter("xT", [IN, npad], f32, isOutput=False)
    idx_pt = nc.declare_dram_parameter("idx_pt", [128, ntiles], i32, isOutput=False)
    ones_pt = nc.declare_dram_parameter("ones_pt", [128, ones_cols], f16, isOutput=False)
    W1p = nc.declare_dram_parameter("W1p", [IN, D1], f32, isOutput=False)
    W1aux = nc.declare_dram_parameter("W1aux", [IN, 16], f32, isOutput=False)
    W2all = nc.declare_dram_parameter("W2all", [D1, 66], f16, isOutput=False)
    b1p = nc.declare_dram_parameter("b1p", [128, 1], f32, isOutput=False)
    b2c = nc.declare_dram_parameter("b2c", [64, 1], f32, isOutput=False)
    exp8 = nc.declare_dram_parameter("exp8", [8, 128], f32, isOutput=False)
    ones1 = nc.declare_dram_parameter("ones1", [1, OUT], f32, isOutput=False)
    out2T = nc.declare_dram_parameter("out2T", [OUT, npad], f32, isOutput=True)

    haug1_sh = nc.dram_tensor("haug1_sh", [npad, EW1], f16)
    haug1 = nc.dram_tensor("haug1", [NT1, EW1], f16, addr_space="Shared")
    haug2_sh = nc.dram_tensor("haug2_sh", [npad, EW2], f16)
    haug2 = nc.dram_tensor("haug2", [NT1, EW2], f16, addr_space="Shared")
    adst1_n = nc.dram_tensor("adst1_n", [npad, 8], f16)
    adst1_s = nc.dram_tensor("adst1_s", [nslots, 8], f16)
    adst2_n = nc.dram_tensor("adst2_n", [npad, 1], f16)
    adst2_s = nc.dram_tensor("adst2_s", [nslots, 1], f16)

    with tile.TileContext(nc) as tc, ExitStack() as ctx:
        sb = ctx.enter_context(tc.tile_pool(name="sb", bufs=2))
        single = ctx.enter_context(tc.tile_pool(name="single", bufs=1))
        gap = ctx.enter_context(tc.tile_pool(name="gap", bufs=3))
        msgp = ctx.enter_context(tc.tile_pool(name="msgp", bufs=2))
        stg = ctx.enter_context(tc.tile_pool(name="stg", bufs=3))

        w1p_t = single.tile([IN, D1], f32)
        nc.sync.dma_start(out=w1p_t[:], in_=W1p[:, :])
        w1x_t = single.tile([IN, 16], f32)
        nc.sync.dma_start(out=w1x_t[:], in_=W1aux[:, :])
        w2_t = single.tile([D1, 66], f16)
        nc.sync.dma_start(out=w2_t[:], in_=W2all[:, :])
        b1_t = single.tile([128, 1], f32)
        nc.sync.dma_start(out=b1_t[:], in_=b1p[:, :])
        b2_t = single.tile([64, 1], f32)
        nc.sync.dma_start(out=b2_t[:], in_=b2c[:, :])
        e8_t = single.tile([8, 128], f32)
        nc.sync.dma_start(out=e8_t[:], in_=exp8[:, :])
        o1_t = single.tile([1, OUT], f32)
        nc.sync.dma_start(out=o1_t[:], in_=ones1[:, :])
        zeros_t = single.tile([128, 512], f16)
        nc.vector.memset(zeros_t[:], 0.0)

        # zero-fill adst slot arrays (dead slots must read finite values)
        def zero_fill(dst_flat, total):
            CH = 128 * 512
            off = 0
            while off < total:
                n = min(CH, total - off)
                cols = n // 128
                nc.sync.dma_start(out=dst_flat[off:off + n],
                                  in_=zeros_t[:, 0:cols])
                off += n
        zero_fill(adst1_s[:, :].rearrange("n e -> (n e)"), nslots * 8)
        zero_fill(adst2_s[:, :].rearrange("n e -> (n e)"), nslots)

        # ------------------------------------------------------------------
        # phase 0
        # ------------------------------------------------------------------
        with tc.tile_pool(name="ps0a", bufs=2, space="PSUM") as ps0a, \
             tc.tile_pool(name="ps0b", bufs=2, space="PSUM") as ps0b:
            for ck in range(npad // 512):
                xt = sb.tile([IN, 512], f32, tag="xt")
                nc.sync.dma_start(out=xt[:], in_=xT[:, ck * 512:(ck + 1) * 512])
                ph = ps0a.tile([128, 512], f32, tag="p0h")
                nc.tensor.matmul(out=ph[:], lhsT=w1p_t[:], rhs=xt[:],
                                 start=True, stop=True)
                px = ps0b.tile([16, 512], f32, tag="p0x")
                nc.tensor.matmul(out=px[:], lhsT=w1x_t[:], rhs=xt[:],
                                 start=True, stop=True)
                sh = stg.tile([128, 512], f16, tag="p0hs")
                nc.scalar.activation(sh[:], ph[:], AF.Copy)
                sx = stg.tile([16, 512], f16, tag="p0xs")
                nc.vector.tensor_copy(out=sx[:], in_=px[:])
                for q in range(4):
                    nb = ck * 512 + q * 128
                    ht = stg.tile([128, 128], f16, tag="p0ht")
                    nc.sync.dma_start(out=ht[:], in_=sh[:, q * 128:(q + 1) * 128],
                                      transpose=True)
                    nc.sync.dma_start(out=haug1_sh[nb:nb + 128, 0:128], in_=ht[:])
                    at = stg.tile([128, 16], f16, tag="p0at")
                    nc.sync.dma_start(out=at[:], in_=sx[:, q * 128:(q + 1) * 128],
                                      transpose=True)
                    nc.sync.dma_start(out=haug1_sh[nb:nb + 128, 128:136],
                                      in_=at[:, 0:8])
                    nc.sync.dma_start(out=adst1_n[nb:nb + 128, :], in_=at[:, 8:16])

        tc.strict_bb_all_engine_barrier()
        nc.gpsimd.collective_compute(
            "AllGather", mybir.AluOpType.bypass,
            ins=[haug1_sh[:, :]], outs=[haug1[:, :]],
            replica_groups=[list(range(NCORES))],
        )
        # adst1 node->slot expansion
        slot_base = 0
        for s in st["sched"]:
            D, m, nt = s["D"], s["m"], s["ntiles"]
            if m * D == 128:
                sap = bass.AP(tensor=adst1_n[:, :].tensor,
                              offset=s["node_base"] * 8,
                              ap=[[8, nt * m], [0, D], [1, 8]])
                nc.sync.dma_start(out=adst1_s[slot_base:slot_base + nt * 128, :],
                                  in_=sap)
            else:
                for t in range(nt):
                    sap = bass.AP(tensor=adst1_n[:, :].tensor,
                                  offset=(s["node_base"] + t * m) * 8,
                                  ap=[[8, m], [0, D], [1, 8]])
                    nc.sync.dma_start(
                        out=adst1_s[slot_base + t * 128:
                                    slot_base + t * 128 + m * D, :],
                        in_=sap)
            slot_base += nt * 128
        tc.strict_bb_all_engine_barrier()

        # ------------------------------------------------------------------
        # edge layers
        # ------------------------------------------------------------------
        def edge_layer(layer):
            if layer == 1:
                table, ew, hw, heads = haug1, EW1, 128, 8
                adst_s, aw = adst1_s, 8
            else:
                table, ew, hw, heads = haug2, EW2, 64, 1
                adst_s, aw = adst2_s, 1

            with tc.tile_pool(name=f"pe{layer}a", bufs=2, space="PSUM") as pA, \
                 tc.tile_pool(name=f"pe{layer}b", bufs=2, space="PSUM") as pB, \
                 tc.tile_pool(name=f"pe{layer}c", bufs=2, space="PSUM") as pC:

                state = dict(cur=None, g_off=0, gi=0, tile_i=0)

                def open_group():
                    if layer == 1:
                        return (pA.tile([128, 128], f32, tag="agg_h", name="agg_h"),
                                pB.tile([8, 128], f32, tag="agg_d", name="agg_d"))
                    return (pA.tile([65, 128], f32, tag="agg2", name="agg2"),)

                def close_group(cur, g):
                    ncols = g["ncols"]
                    gb = g["node_base"]
                    if layer == 1:
                        p_h, p_d = cur
                        rec = stg.tile([8, 128], f32, tag="rec")
                        nc.vector.tensor_scalar(out=rec[:, :ncols], in0=p_d[:, :ncols],
                                                scalar1=1e-30, scalar2=None, op0=OP.add)
                        nc.vector.reciprocal(out=rec[:, :ncols], in_=rec[:, :ncols])
                        p_r = pC.tile([128, 128], f32, tag="recx")
                        nc.tensor.matmul(out=p_r[:, :ncols], lhsT=e8_t[:],
                                         rhs=rec[:, :ncols], start=True, stop=True)
                        rsb = stg.tile([128, 128], f32, tag="recsb")
                        nc.scalar.activation(rsb[:, :ncols], p_r[:, :ncols], AF.Copy)
                        xn = stg.tile([128, 128], f16, tag="xn")
                        nc.vector.tensor_tensor(out=xn[:, :ncols], in0=p_h[:, :ncols],
                                                in1=rsb[:, :ncols], op=OP.mult)
                        m_ = stg.tile([128, 128], f16, tag="elm")
                        nc.vector.tensor_scalar(out=m_[:, :ncols], in0=xn[:, :ncols],
                                                scalar1=b1_t[:], scalar2=0.0,
                                                op0=OP.add, op1=OP.min)
                        r_ = stg.tile([128, 128], f16, tag="elr")
                        nc.vector.tensor_scalar(out=r_[:, :ncols], in0=xn[:, :ncols],
                                                scalar1=b1_t[:], scalar2=0.0,
                                                op0=OP.add, op1=OP.max)
                        nc.scalar.activation(m_[:, :ncols], m_[:, :ncols], AF.Exp)
                        h2 = stg.tile([128, 128], f16, tag="h2")
                        nc.vector.scalar_tensor_tensor(
                            out=h2[:, :ncols], in0=m_[:, :ncols], scalar=-1.0,
                            in1=r_[:, :ncols], op0=OP.add, op1=OP.add)
                        p_g = pC.tile([66, 128], f32, tag="aug")
                        nc.tensor.matmul(out=p_g[:, :ncols], lhsT=w2_t[:],
                                         rhs=h2[:, :ncols], start=True, stop=True)
                        s80 = stg.tile([80, 128], f16, tag="s80")
                        nc.scalar.activation(s80[0:66, :], p_g[:, :], AF.Copy)
                        st_t = stg.tile([128, 80], f16, tag="stt80")
                        nc.sync.dma_start(out=st_t[:], in_=s80[:], transpose=True)
                        nc.sync.dma_start(out=haug2_sh[gb:gb + ncols, 0:EW2],
                                          in_=st_t[:ncols, 0:EW2])
                        nc.sync.dma_start(out=adst2_n[gb:gb + ncols, 0:1],
                                          in_=s80[64:66, :ncols][1:2, :])
                    else:
                        (p,) = cur
                        rec = stg.tile([1, 128], f32, tag="rec2")
                        nc.vector.tensor_scalar(out=rec[:, :ncols], in0=p[64:65, :ncols],
                                                scalar1=1e-30, scalar2=None, op0=OP.add)
                        nc.vector.reciprocal(out=rec[:, :ncols], in_=rec[:, :ncols])
                        p_r = pC.tile([64, 128], f32, tag="recx2")
                        nc.tensor.matmul(out=p_r[:, :ncols], lhsT=o1_t[:],
                                         rhs=rec[:, :ncols], start=True, stop=True)
                        rsb = stg.tile([64, 128], f32, tag="recsb2")
                        nc.scalar.activation(rsb[:, :ncols], p_r[:, :ncols], AF.Copy)
                        o = stg.tile([64, 128], f32, tag="o2")
                        nc.vector.tensor_tensor(out=o[:, :ncols], in0=p[0:64, :ncols],
                                                in1=rsb[:, :ncols], op=OP.mult)
                        nc.vector.tensor_scalar(out=o[:, :ncols], in0=o[:, :ncols],
                                                scalar1=b2_t[:], scalar2=None,
                                                op0=OP.add)
                        nc.sync.dma_start(out=out2T[:, gb:gb + ncols],
                                          in_=o[:, :ncols])

                nbatches = -(-ntiles // KB)
                for b in range(nbatches):
                    k = min(KB, ntiles - b * KB)
                    idx_t = sb.tile([128, KB], i32, tag="idx")
                    nc.sync.dma_start(out=idx_t[:, :k],
                                      in_=idx_pt[:, b * KB:b * KB + k])
                    ga = gap.tile([128, KB, ew], f16, tag="ga")
                    for t in range(k):
                        nc.gpsimd.indirect_dma_start(
                            out=ga[:, t, :], out_offset=None, in_=table[:, :],
                            in_offset=bass.IndirectOffsetOnAxis(
                                ap=idx_t[:, t:t + 1], axis=0),
                        )
                    adt = sb.tile([128, KB, aw], f16, tag="adt")
                    sap = bass.AP(tensor=adst_s[:, :].tensor,
                                  offset=b * KB * 128 * aw,
                                  ap=[[aw, 128], [128 * aw, k], [1, aw]])
                    nc.sync.dma_start(out=adt[:, :k, :], in_=sap)
                    adt2 = sb.tile([128, KB, aw], f16, tag="adt2")
                    nc.vector.tensor_copy(out=adt2[:, :k, :], in_=adt[:, :k, :])
                    msg = msgp.tile([128, KB, ew], f16, tag="msg")
                    nc.vector.tensor_tensor(out=msg[:, :k, hw:hw + heads],
                                            in0=ga[:, :k, hw:hw + heads],
                                            in1=adt2[:, :k, :], op=OP.add)
                    nc.vector.scalar_tensor_tensor(
                        out=msg[:, :k, hw:hw + heads],
                        in0=msg[:, :k, hw:hw + heads], scalar=NEG_SLOPE,
                        in1=msg[:, :k, hw:hw + heads],
                        op0=OP.mult, op1=OP.max)
                    nc.scalar.activation(msg[:, :k, hw:hw + heads],
                                         msg[:, :k, hw:hw + heads], AF.Exp)
                    exap = msg[:, :k, hw:hw + heads]
                    if layer == 1:
                        ga_v = ga[:, :k, 0:hw].rearrange("p k (c h) -> p k c h",
                                                         h=heads)
                        ex_b = bass.AP(tensor=exap.tensor, offset=exap.offset,
                                       ap=[exap.ap[0], [ew, k], [0, C1], [1, heads]])
                        msg_v = msg[:, :k, 0:hw].rearrange("p k (c h) -> p k c h",
                                                           h=heads)
                    else:
                        ga_v = ga[:, :k, 0:hw]
                        ex_b = bass.AP(tensor=exap.tensor, offset=exap.offset,
                                       ap=[exap.ap[0], [ew, k], [0, hw]])
                        msg_v = msg[:, :k, 0:hw]
                    nc.vector.tensor_tensor(out=msg_v, in0=ga_v, in1=ex_b,
                                            op=OP.mult)

                    cb0 = int(col_of_tile[b * KB])
                    cb1 = int(col_of_tile[b * KB + k])
                    ones_t = sb.tile([128, 64 * KB], f16, tag="ones")
                    nc.sync.dma_start(out=ones_t[:, 0:cb1 - cb0],
                                      in_=ones_pt[:, cb0:cb1])

                    for t in range(k):
                        g = groups[state["gi"]]
                        if state["cur"] is None:
                            state["cur"] = open_group()
                        m = g["m"]
                        oc = int(col_of_tile[state["tile_i"]]) - cb0
                        go = state["g_off"]
                        if layer == 1:
                            p_h, p_d = state["cur"]
                            nc.tensor.matmul(out=p_h[:, go:go + m],
                                             lhsT=msg[:, t, 0:hw],
                                             rhs=ones_t[:, oc:oc + m],
                                             start=True, stop=True)
                            nc.tensor.matmul(out=p_d[:, go:go + m],
                                             lhsT=msg[:, t, hw:hw + heads],
                                             rhs=ones_t[:, oc:oc + m],
                                             start=True, stop=True)
                        else:
                            (p,) = state["cur"]
                            nc.tensor.matmul(out=p[:, go:go + m],
                                             lhsT=msg[:, t, 0:hw + 1],
                                             rhs=ones_t[:, oc:oc + m],
                                             start=True, stop=True)
                        state["g_off"] += m
                        state["tile_i"] += 1
                        if state["g_off"] >= g["ncols"]:
                            close_group(state["cur"], g)
                            state["cur"] = None
                            state["g_off"] = 0
                            state["gi"] += 1

                assert state["tile_i"] == ntiles and state["gi"] == len(groups)

        edge_layer(1)

        tc.strict_bb_all_engine_barrier()
        nc.gpsimd.collective_compute(
            "AllGather", mybir.AluOpType.bypass,
            ins=[haug2_sh[:, :]], outs=[haug2[:, :]],
            replica_groups=[list(range(NCORES))],
        )
        slot_base = 0
        for s in st["sched"]:
            D, m, nt = s["D"], s["m"], s["ntiles"]
            if m * D == 128:
                sap = bass.AP(tensor=adst2_n[:, :].tensor, offset=s["node_base"],
                              ap=[[1, nt * m], [0, D], [1, 1]])
                nc.sync.dma_start(out=adst2_s[slot_base:slot_base + nt * 128, :],
                                  in_=sap)
            else:
                for t in range(nt):
                    sap = bass.AP(tensor=adst2_n[:, :].tensor,
                                  offset=s["node_base"] + t * m,
                                  ap=[[1, m], [0, D], [1, 1]])
                    nc.sync.dma_start(
                        out=adst2_s[slot_base + t * 128:
                                    slot_base + t * 128 + m * D, :],
                        in_=sap)
            slot_base += nt * 128
        tc.strict_bb_all_engine_barrier()

        edge_layer(2)

    nc.compile()
    return nc


_CACHE = {}


def kernel(x, edge_index, W1, a1_src, a1_dst, b1, W2, a2_src, a2_dst, b2,
           _return_extras=False, _trace=False):
    from concourse.bass_utils import run_bass_kernel_spmd

    x = np.asarray(x)
    edge_index = np.asarray(edge_index)
    st = _build_structure(edge_index)
    consts, pca = _host_arrays(st, x, W1, a1_src, a1_dst, b1, W2,
                               a2_src, a2_dst, b2)

    if "k" not in _CACHE:
        _CACHE["k"] = _build_kernel(st)
    nc = _CACHE["k"]

    in_maps = []
    for c in range(NCORES):
        in_maps.append(dict(
            xT=pca[c]["xT"], idx_pt=pca[c]["idx_pt"], ones_pt=pca[c]["ones_pt"],
            W1p=consts["W1p"], W1aux=consts["W1aux"], W2all=consts["W2all"],
            b1p=consts["b1p"], b2c=consts["b2c"],
            exp8=consts["exp8"], ones1=consts["ones1"],
        ))
    kw = {}
    if _trace:
        kw = dict(trace=True)
    res = run_bass_kernel_spmd(nc, in_maps, list(range(NCORES)), **kw)

    out = np.zeros((N, OUT), np.float32)
    for c in range(NCORES):
        o = res.results[c]["out2T"]
        perm_rows = st["per_core"][c]["perm_rows"]
        valid = perm_rows >= 0
        out[perm_rows[valid]] = o.T[valid]
    if _return_extras:
        return out, res
    return out
